# revision 24
# baseline (speedup 1.0000x reference)
"""HMQ-quantized MLP (fc1 -> exact GELU -> fc2) on 8 TRN2 NeuronCores.

Strategy: data-parallel over the 16384 token rows (2048 rows/core).
The int8 fake-quant values are integers in [-127, 127], exactly representable
in bf16, and all dot-product partial sums stay far below 2^24 -- so the
dequantized GEMMs are computed EXACTLY as bf16 integer matmuls on the PE
array with fp32 PSUM accumulation, then scaled by s_a*s_w.  Rounding uses
the +/-1.5*2^23 magic-constant trick (matches jnp.round round-half-even).

All quantization reads f32 sources (any bf16 pre-rounding of continuous
values flips ~3% of the round-to-int decisions and costs ~1% rel-err per
tensor -- measured, not acceptable).  g stages to DRAM in f32.

Scheduling (v2, trace-driven):
 - x shard stays in SBUF after the absmax pass (no re-read); the absmax
   DMA stream rides the sync queue alone so the scale doorbell rings
   ~40us in, BEFORE the framework's startup barrier releases -- AllGather
   #1 then starts the moment the barrier ends.
 - x quantize fans across Scalar AND Pool engines in matmul consumption
   order while DVE slices w1 batch-0 per-ic; fc1's first two windows run
   ic-outer so each xq chunk is consumed for both oc halves back-to-back
   (PE never outruns the quantizers).
 - w2 load+quant finishes by window 12 so fc1's tail has no bulk DMA in
   flight: the 16-byte g-scale doorbell then completes in ~1us instead of
   queueing ~7us behind gT writes.
 - g absmax partials are pre-combined at window 14; after the last gelu
   only an 8-column reduce + cross-partition max remain before the
   AllGather #2 trigger.
 - collective readbacks ride the scalar HWDGE queue; the first fc2 gq
   tiles quantize in quarters alternating Scalar/Vector so fc2's first
   LDWEIGHTS issues ~4us after the collective lands.
"""

import numpy as np

import concourse.bass as bass
import concourse.mybir as mybir
import concourse.tile as tile
from concourse import bacc, bass_isa
from concourse.bass_utils import run_bass_kernel_spmd

F32 = mybir.dt.float32
BF16 = mybir.dt.bfloat16
ts = bass.ts

C_MAGIC = 1.5 * 2**23  # round-to-nearest-even for |v| < 2^22
QMAX = 127.0

NCORES = 8
B, T, D, H = 4, 4096, 1024, 4096
M = B * T            # 16384 total rows
S = M // NCORES      # 2048 rows per core

N_IC = D // 128      # 8  contraction chunks for fc1
N_OC = H // 128      # 32 output chunks for fc1 (hidden)
N_ST = S // 512      # 4  row tiles of 512
N_SC = S // 128      # 16 row chunks of 128
N_NC = H // 128      # 32 contraction chunks for fc2
N_JT = D // 512      # 2  output col tiles for fc2

Copy = mybir.ActivationFunctionType.Copy
Gelu = mybir.ActivationFunctionType.Gelu
X_AX = mybir.AxisListType.X
MAX = mybir.AluOpType.max
MULT = mybir.AluOpType.mult
SUB = mybir.AluOpType.subtract
ADD = mybir.AluOpType.add

# w2 chunk quantization schedule: all 32 chunks done by window 12 so the
# fc1 tail carries no bulk DMA (chunk 0 is prefetched at startup).
W2SCHED = {}
_hc = 0
for _w in range(1, 13):
    _n = 3 if _w <= 8 else 2
    W2SCHED[_w] = list(range(_hc, _hc + _n))
    _hc += _n
assert _hc == 32


def build():
    nc = bacc.Bacc("TRN2", target_bir_lowering=False, debug=False,
                   num_devices=NCORES)

    xts = nc.dram_tensor("xts", [D, S], F32, kind="ExternalInput")
    w1r = nc.dram_tensor("w1r", [128, N_IC, H], F32, kind="ExternalInput")
    w1s = nc.dram_tensor("w1s", [H // NCORES, D], F32, kind="ExternalInput")
    w2t = nc.dram_tensor("w2t", [H, D], F32, kind="ExternalInput")
    w2s = nc.dram_tensor("w2s", [D // NCORES, H], F32, kind="ExternalInput")
    b1a = nc.dram_tensor("b1a", [128, N_OC], F32, kind="ExternalInput")
    b2m = nc.dram_tensor("b2m", [128, D], F32, kind="ExternalInput")
    out = nc.dram_tensor("out", [S, D], F32, kind="ExternalOutput")

    with tile.TileContext(nc) as tc:
        with (
            tc.tile_pool(name="misc", bufs=1) as misc,
            tc.tile_pool(name="big", bufs=1) as bigp,
            tc.tile_pool(name="xq", bufs=1) as xqp,
            tc.tile_pool(name="w1c", bufs=3) as w1sp,
            tc.tile_pool(name="w1q", bufs=2) as w1qp,
            tc.tile_pool(name="w2c", bufs=3) as w2sp,
            tc.tile_pool(name="gout", bufs=3) as goutp,
            tc.tile_pool(name="gts", bufs=2) as gtsp,
            tc.tile_pool(name="gq", bufs=2) as gqp,
            tc.tile_pool(name="outp", bufs=2) as outp,
            tc.tile_pool(name="psum", bufs=8, space="PSUM") as psump,
            tc.tile_pool(name="dram", bufs=1, space="DRAM") as dramp,
        ):
            # ---------------- persistent DRAM intermediates ----------------
            gT = dramp.tile([H, S], F32, tag="gT")
            cc1_in = dramp.tile([1, 4], F32, tag="cc1i")
            cc1_out = dramp.tile([NCORES, 4], F32, tag="cc1o")
            cc2_in = dramp.tile([1, 4], F32, tag="cc2i")
            cc2_out = dramp.tile([NCORES, 4], F32, tag="cc2o")

            # b1 blocked on host; small, rides the scalar HWDGE queue early
            b1sb = misc.tile([128, N_OC], F32, tag="b1sb")
            nc.scalar.dma_start(out=b1sb, in_=b1a[:, :])

            # garow cols 1..3 are never meaningful; memset once so the
            # cross-partition reduce doesn't read uninitialized SBUF.
            garow = misc.tile([128, 4], F32, tag="garow")
            nc.vector.memset(garow, 0.0)

            # dummy Gelu up front: the ACT table set covering Gelu+Copy
            # loads now (~1.3us) instead of inside fc1's first window
            nc.scalar.activation(garow[:, 3:4], garow[:, 3:4], Gelu,
                                 bias=0.0, scale=1.0)

            # ------------- local abs-max pass; x STAYS in SBUF -------------
            # All absmax DMAs ride the sync queue back-to-back (358 GB/s).
            # W shards go FIRST: their reduces (incl. the slower XY form and
            # the ws-pool rotation) then hide under the x DMA stream, and
            # the critical path ends with the last x reduce chasing its DMA.
            # part1 cols: 0..7 x | 8..9 w1 | 10..11 w2 | 12..13 x7 halves
            xf = bigp.tile([128, N_IC, S], F32, tag="big", name="xf32")
            part1 = misc.tile([128, 14], F32, tag="part1")
            # w1 shard [512, 1024] -> 2 chunks [128, 2, 1024]
            # (w-chunks borrow the gout staging buffers: same size, and the
            # staging lifetime starts long after the absmax pass retires)
            for c in range(2):
                wc = goutp.tile([128, 2, 1024], F32, tag="gout",
                                name=f"w1m{c}")
                nc.sync.dma_start(
                    out=wc,
                    in_=w1s[c * 256:(c + 1) * 256, :].rearrange(
                        "(a p) d -> p a d", p=128))
                nc.vector.tensor_reduce(out=part1[:, 8 + c:9 + c], in_=wc,
                                        axis=mybir.AxisListType.XY,
                                        op=MAX, apply_absolute_value=True)
            # w2 shard [128, 4096] -> 2 chunks [128, 2048]
            for c in range(2):
                wc = goutp.tile([128, 2048], F32, tag="gout",
                                name=f"w2m{c}")
                nc.sync.dma_start(out=wc,
                                  in_=w2s[:, c * 2048:(c + 1) * 2048])
                nc.vector.tensor_reduce(out=part1[:, 10 + c:11 + c], in_=wc,
                                        axis=X_AX,
                                        op=MAX, apply_absolute_value=True)
            for ic in range(N_IC - 1):
                nc.sync.dma_start(out=xf[:, ic, :],
                                  in_=xts[ic * 128:(ic + 1) * 128, :])
                nc.vector.tensor_reduce(out=part1[:, ic:ic + 1],
                                        in_=xf[:, ic, :], axis=X_AX,
                                        op=MAX, apply_absolute_value=True)
            # last chunk lands and reduces in halves: shorter reduce tail
            lc = N_IC - 1
            for hf in range(2):
                nc.sync.dma_start(
                    out=xf[:, lc, hf * 1024:(hf + 1) * 1024],
                    in_=xts[lc * 128:(lc + 1) * 128,
                            hf * 1024:(hf + 1) * 1024])
                nc.vector.tensor_reduce(
                    out=part1[:, 12 + hf:13 + hf],
                    in_=xf[:, lc, hf * 1024:(hf + 1) * 1024], axis=X_AX,
                    op=MAX, apply_absolute_value=True)
            nc.vector.tensor_reduce(out=part1[:, 7:8], in_=part1[:, 12:14],
                                    axis=X_AX, op=MAX)

            # combine partials -> [x, w1, w2, w2] cols of arow
            arow = misc.tile([128, 4], F32, tag="arow")
            nc.vector.tensor_reduce(out=arow[:, 0:1], in_=part1[:, 0:8],
                                    axis=X_AX, op=MAX)
            nc.vector.tensor_reduce(out=arow[:, 1:2], in_=part1[:, 8:10],
                                    axis=X_AX, op=MAX)
            nc.vector.tensor_reduce(out=arow[:, 2:3], in_=part1[:, 10:12],
                                    axis=X_AX, op=MAX)
            nc.vector.tensor_copy(arow[:, 3:4], arow[:, 2:3])
            armax = misc.tile([128, 4], F32, tag="armax")
            nc.gpsimd.partition_all_reduce(armax, arow, channels=128,
                                           reduce_op=bass_isa.ReduceOp.max)

            # ------------- AllGather #1 -> global Mx, Mw1, Mw2 -------------
            # Doorbell rings ~50us in, before the startup barrier releases;
            # the AG then starts the moment the barrier ends.  The trigger
            # payload rides the idle scalar HWDGE queue so its completion
            # (which the doorbell write waits on) is ~1us, never queued
            # behind bulk traffic.
            nc.scalar.dma_start(out=cc1_in, in_=armax[0:1, :])
            nc.gpsimd.collective_compute(
                "AllGather", mybir.AluOpType.bypass,
                replica_groups=[list(range(NCORES))],
                ins=[cc1_in.opt()], outs=[cc1_out.opt()])

            # Startup prefetch rides the GPSIMD queue gated past the
            # doorbell (the tile scheduler would otherwise hoist these 3MB
            # ahead of the absmax stream and starve it -- measured).
            w1cs = []
            w2pre = {}
            with tc.tile_wait_until(0.054):
                for ocb in range(2):
                    w1c = w1sp.tile([128, N_IC, 256], F32, tag="w1c",
                                    name=f"w1c{ocb}")
                    nc.gpsimd.dma_start(
                        out=w1c, in_=w1r[:, :, ocb * 256:(ocb + 1) * 256])
                    w1cs.append(w1c)
                w2c0 = w2sp.tile([128, D], F32, tag="w2c", name="w2c0")
                nc.gpsimd.dma_start(out=w2c0, in_=w2t[0:128, :])
                w2pre[0] = w2c0
                b2r = misc.tile([128, D], F32, tag="b2r")
                nc.gpsimd.dma_start(out=b2r, in_=b2m[:, :])

            # readback on the scalar HWDGE queue (ACT idle until quantize)
            g1g = misc.tile([NCORES, 4], F32, tag="g1g")
            nc.scalar.dma_start(out=g1g, in_=cc1_out[:, :])
            g1m = misc.tile([NCORES, 4], F32, tag="g1m")
            nc.gpsimd.partition_all_reduce(g1m, g1g, channels=NCORES,
                                           reduce_op=bass_isa.ReduceOp.max)
            g1 = misc.tile([128, 4], F32, tag="g1")
            nc.gpsimd.partition_broadcast(g1, g1m)

            # scl cols: 0 sx | 1 inv_sx | 2 sw1 | 3 inv_sw1 | 4 sw2 |
            #           5 inv_sw2 | 6 d1
            scl = misc.tile([128, 8], F32, tag="scl")
            for i in range(3):
                nc.vector.tensor_scalar(out=scl[:, 2 * i:2 * i + 1],
                                        in0=g1[:, i:i + 1],
                                        scalar1=1e-8, scalar2=1.0 / QMAX,
                                        op0=MAX, op1=MULT)
                nc.vector.reciprocal(scl[:, 2 * i + 1:2 * i + 2],
                                     scl[:, 2 * i:2 * i + 1])
            nc.vector.tensor_mul(scl[:, 6:7], scl[:, 0:1], scl[:, 2:3])

            # ---- quantize: 3-engine split in fc1 consumption order --------
            # Per ic chunk: DVE slices w1 batch-0; ACT does the h0 half
            # (both magic-round passes); Pool does the h1 multiply-add
            # (f32->f32 runs at line rate there; its bf16-cast SUB is
            # software-slow -- measured 14.7us -- so DVE finishes h1).
            # w1 batch-1 quantizes on DVE mid-stream, ready before window 1.
            w1q0 = w1qp.tile([128, N_IC, 256], BF16, tag="w1q", name="w1q0")
            xqT = xqp.tile([128, N_IC, S], BF16, tag="xq")
            w1q1 = w1qp.tile([128, N_IC, 256], BF16, tag="w1q", name="w1q1")
            for ic in range(N_IC):
                # DVE: w1 batch-0 slice + x h1 finish
                w1cf = w1cs[0][:, ic, :]
                nc.vector.tensor_scalar(out=w1cf, in0=w1cf,
                                        scalar1=scl[:, 3:4], scalar2=C_MAGIC,
                                        op0=MULT, op1=ADD)
                nc.vector.tensor_scalar(out=w1q0[:, ic, :], in0=w1cf,
                                        scalar1=C_MAGIC, scalar2=None,
                                        op0=SUB)
                nsp = 2 if ic == 0 else 1  # split ic0-h0 for a fast first MM
                for q in range(nsp):
                    w = 1024 // nsp
                    xh = xf[:, ic, q * w:(q + 1) * w]
                    qh = xqT[:, ic, q * w:(q + 1) * w]
                    nc.scalar.activation(xh, xh, Copy,
                                         bias=C_MAGIC, scale=scl[:, 1:2])
                    nc.scalar.activation(qh, xh, Copy,
                                         bias=-C_MAGIC, scale=1.0)
                # ACT also quantizes w1 batch-1 per-ic (it has headroom;
                # a bulk DVE pass here stalled the window-0 xq cadence)
                w1c1f = w1cs[1][:, ic, :]
                nc.scalar.activation(w1c1f, w1c1f, Copy,
                                     bias=C_MAGIC, scale=scl[:, 3:4])
                nc.scalar.activation(w1q1[:, ic, :], w1c1f, Copy,
                                     bias=-C_MAGIC, scale=1.0)
                xh = xf[:, ic, 1024:2048]
                qh = xqT[:, ic, 1024:2048]
                nc.gpsimd.tensor_scalar(out=xh, in0=xh,
                                        scalar1=scl[:, 1:2],
                                        scalar2=C_MAGIC,
                                        op0=MULT, op1=ADD)
                nc.vector.tensor_scalar(out=qh, in0=xh,
                                        scalar1=C_MAGIC, scalar2=None,
                                        op0=SUB)

            def quant_w1(ocb):
                # bulk DVE quantize for batches 1..15 (batch 0 was sliced)
                w1c = w1cs[ocb]
                w1q = w1qp.tile([128, N_IC, 256], BF16, tag="w1q",
                                name=f"w1q{ocb}")
                w1cf = w1c.rearrange("p a b -> p (a b)")
                nc.vector.tensor_scalar(out=w1cf, in0=w1cf,
                                        scalar1=scl[:, 3:4], scalar2=C_MAGIC,
                                        op0=MULT, op1=ADD)
                nc.vector.tensor_scalar(
                    out=w1q.rearrange("p a b -> p (a b)"), in0=w1cf,
                    scalar1=C_MAGIC, scalar2=None, op0=SUB)
                return w1q

            w1qs = [w1q0, w1q1]

            # ---------------- fc1: h^T = w1q @ xq^T, gelu, stage g^T -------
            # w2qT reuses xf's SBUF slot (same pool tag, same byte size);
            # its first write WARs on the last x-quant read.
            w2qT = bigp.tile([128, N_NC, D], BF16, tag="big", name="w2qT")
            # gpart cols 0..29: one absmax per oc 0..29 (single [128,2048]
            # reduce per staged oc); cols 32..39: per-st partials for oc
            # 30/31 so the doorbell chain overlaps the last gelu drain.
            gpart = misc.tile([128, 40], F32, tag="gpart")
            gpre = misc.tile([128, 2], F32, tag="gpre")
            gstages = []
            for ocb in range(N_OC // 2):
                if ocb + 2 <= N_OC // 2 - 1:
                    w1n = w1sp.tile([128, N_IC, 256], F32, tag="w1c",
                                    name=f"w1c{ocb + 2}")
                    # windows 0/1: gate the prefetch past the doorbell so
                    # it can't steal absmax-stream bandwidth
                    with tc.tile_wait_until(0.055, enable=(ocb < 2)):
                        nc.sync.dma_start(
                            out=w1n,
                            in_=w1r[:, :, (ocb + 2) * 256:(ocb + 3) * 256])
                    w1cs.append(w1n)
                if 2 <= ocb + 1 <= N_OC // 2 - 1:
                    w1qs.append(quant_w1(ocb + 1))
                # w2 load+quant, all done by window 12; the f32 multiply-add
                # pass rides the otherwise-idle Pool engine
                for hc in W2SCHED.get(ocb, ()):
                    if hc in w2pre:
                        w2c = w2pre[hc]
                    else:
                        w2c = w2sp.tile([128, D], F32, tag="w2c",
                                        name=f"w2c{hc}")
                        with tc.tile_wait_until(0.055, enable=(ocb < 2)):
                            nc.sync.dma_start(out=w2c,
                                              in_=w2t[ts(hc, 128), :])
                    nc.gpsimd.tensor_scalar(out=w2c, in0=w2c,
                                            scalar1=scl[:, 5:6],
                                            scalar2=C_MAGIC,
                                            op0=MULT, op1=ADD)
                    nc.vector.tensor_scalar(out=w2qT[:, hc, :], in0=w2c,
                                            scalar1=C_MAGIC,
                                            scalar2=None, op0=SUB)
                w1q = w1qs[ocb]
                pts = [[psump.tile([128, 512], F32, tag="mm",
                                   name=f"pt{2 * ocb + j}_{st}")
                        for st in range(N_ST)] for j in range(2)]
                if ocb < 2:
                    # ic-outer: each xq chunk feeds both oc halves
                    # back-to-back, so quantization keeps pace with the PE.
                    # Within an ic, the h0-half row tiles (st 0/1, produced
                    # by ACT) run before the h1 tiles (Pool+DVE, ~1us
                    # later) -- matches production order.
                    for ic in range(N_IC):
                        for half in range(2):
                            for j in range(2):
                                for st in (2 * half, 2 * half + 1):
                                    nc.tensor.matmul(
                                        pts[j][st],
                                        lhsT=w1q[:, ic,
                                                 j * 128:(j + 1) * 128],
                                        rhs=xqT[:, ic, ts(st, 512)],
                                        start=(ic == 0),
                                        stop=(ic == N_IC - 1))
                else:
                    # j-outer: consecutive matmuls share the stationary
                    # operand (deduped LDWEIGHTS stay hidden).
                    for j in range(2):
                        for ic in range(N_IC):
                            for st in range(N_ST):
                                nc.tensor.matmul(
                                    pts[j][st],
                                    lhsT=w1q[:, ic, j * 128:(j + 1) * 128],
                                    rhs=xqT[:, ic, ts(st, 512)],
                                    start=(ic == 0), stop=(ic == N_IC - 1))
                for j in range(2):
                    oc = 2 * ocb + j
                    last = ocb >= N_OC // 2 - 1
                    stg = goutp.tile([128, 2048], F32, tag="gout",
                                     name=f"stg{oc}")
                    for st in range(N_ST):
                        gv = stg[:, st * 512:(st + 1) * 512]
                        nc.scalar.activation(gv, pts[j][st], Gelu,
                                             bias=b1sb[:, oc:oc + 1],
                                             scale=scl[:, 6:7])
                        if last:
                            # per-st partials: the absmax chain overlaps
                            # the final gelu drain
                            col = 32 + 4 * j + st
                            nc.vector.tensor_reduce(
                                out=gpart[:, col:col + 1], in_=gv,
                                axis=X_AX, op=MAX,
                                apply_absolute_value=True)
                    if last:
                        # gT write deferred past the scale doorbell so the
                        # fc1 tail has no bulk DMA in flight when the 16B
                        # trigger payload lands
                        gstages.append((oc, stg))
                    else:
                        nc.vector.tensor_reduce(
                            out=gpart[:, oc:oc + 1], in_=stg,
                            axis=X_AX, op=MAX, apply_absolute_value=True)
                        nc.sync.dma_start(out=gT[ts(oc, 128), :], in_=stg)
                if ocb == N_OC // 2 - 2:
                    # pre-combine oc 0..29 absmax partials during the tail
                    nc.vector.tensor_reduce(out=gpre[:, 0:1],
                                            in_=gpart[:, 0:30],
                                            axis=X_AX, op=MAX)

            # ---------------- AllGather #2 trigger: global Mg --------------
            nc.vector.tensor_reduce(out=gpre[:, 1:2], in_=gpart[:, 32:40],
                                    axis=X_AX, op=MAX)
            nc.vector.tensor_tensor(out=garow[:, 0:1], in0=gpre[:, 0:1],
                                    in1=gpre[:, 1:2], op=MAX)
            gamax = misc.tile([128, 4], F32, tag="gamax")
            nc.gpsimd.partition_all_reduce(gamax, garow, channels=128,
                                           reduce_op=bass_isa.ReduceOp.max)
            # trigger payload on the (now idle) scalar HWDGE queue: its
            # completion gates the doorbell write and must not sit behind
            # the gT-write / gather backlog (costs ~7us there -- measured)
            nc.scalar.dma_start(out=cc2_in, in_=gamax[0:1, :])
            nc.gpsimd.collective_compute(
                "AllGather", mybir.AluOpType.bypass,
                replica_groups=[list(range(NCORES))],
                ins=[cc2_in.opt()], outs=[cc2_out.opt()])

            # deferred last-window gT writes: queued on scalar BEHIND the
            # parked cc2_in trigger, so they execute during the AllGather
            for oc, stg in gstages:
                nc.scalar.dma_start(out=gT[ts(oc, 128), :], in_=stg)

            g2g = misc.tile([NCORES, 4], F32, tag="g2g")
            nc.scalar.dma_start(out=g2g, in_=cc2_out[:, :])
            g2m = misc.tile([NCORES, 4], F32, tag="g2m")
            nc.gpsimd.partition_all_reduce(g2m, g2g, channels=NCORES,
                                           reduce_op=bass_isa.ReduceOp.max)
            g2 = misc.tile([128, 4], F32, tag="g2")
            nc.gpsimd.partition_broadcast(g2, g2m)

            # scl2 cols: 0 sg | 1 inv_sg | 2 d2
            scl2 = misc.tile([128, 4], F32, tag="scl2")
            nc.vector.tensor_scalar(out=scl2[:, 0:1], in0=g2[:, 0:1],
                                    scalar1=1e-8, scalar2=1.0 / QMAX,
                                    op0=MAX, op1=MULT)
            nc.vector.reciprocal(scl2[:, 1:2], scl2[:, 0:1])
            nc.vector.tensor_mul(scl2[:, 2:3], scl2[:, 0:1], scl[:, 4:5])

            # ---------------- fc2: out = gq^T.T @ w2q^T --------------------
            for sc in range(N_SC):
                gqs = []
                for half in range(2):
                    gs = gtsp.tile([128, 16, 128], F32, tag="gts",
                                   name=f"gs{sc}_{half}")
                    nc.sync.dma_start(
                        out=gs,
                        in_=gT[half * 2048:(half + 1) * 2048,
                               ts(sc, 128)].rearrange("(a p) s -> p a s",
                                                      p=128))
                    gq = gqp.tile([128, 16, 128], BF16, tag="gq",
                                  name=f"gq{sc}_{half}")
                    if sc < 2:
                        # fine-grained, alternating engines: the first
                        # lhsT tile is ready well under 1us after scl2.
                        npc = 8 if (sc == 0 and half == 0) else 4
                        w = 16 // npc
                        for q in range(npc):
                            gsf = gs[:, w * q:w * (q + 1), :].rearrange(
                                "p a b -> p (a b)")
                            gqf = gq[:, w * q:w * (q + 1), :].rearrange(
                                "p a b -> p (a b)")
                            nc.scalar.activation(gsf, gsf, Copy,
                                                 bias=C_MAGIC,
                                                 scale=scl2[:, 1:2])
                            nc.vector.tensor_scalar(out=gqf, in0=gsf,
                                                    scalar1=C_MAGIC,
                                                    scalar2=None, op0=SUB)
                    else:
                        gsf = gs.rearrange("p a b -> p (a b)")
                        nc.scalar.activation(gsf, gsf, Copy, bias=C_MAGIC,
                                             scale=scl2[:, 1:2])
                        nc.vector.tensor_scalar(
                            out=gq.rearrange("p a b -> p (a b)"), in0=gsf,
                            scalar1=C_MAGIC, scalar2=None, op0=SUB)
                    gqs.append(gq)
                pos = [psump.tile([128, 512], F32, tag="mm",
                                  name=f"po{sc}_{jt}")
                       for jt in range(N_JT)]
                for nn in range(N_NC):
                    for jt in range(N_JT):
                        nc.tensor.matmul(pos[jt],
                                         lhsT=gqs[nn // 16][:, nn % 16, :],
                                         rhs=w2qT[:, nn, ts(jt, 512)],
                                         start=(nn == 0),
                                         stop=(nn == N_NC - 1))
                # last row-chunk drains in 256-col pieces: shorter kernel
                # tail after the final matmul
                nseg = 2 if sc == N_SC - 1 else 1
                for jt in range(N_JT):
                    ot = outp.tile([128, 512], F32, tag="ot",
                                   name=f"ot{sc}_{jt}")
                    for g in range(nseg):
                        w = 512 // nseg
                        osl = slice(g * w, (g + 1) * w)
                        nc.scalar.activation(ot[:, osl], pos[jt][:, osl],
                                             Copy, bias=0.0,
                                             scale=scl2[:, 2:3])
                        nc.vector.tensor_add(ot[:, osl], ot[:, osl],
                                             b2r[:, jt * 512 + g * w:
                                                 jt * 512 + (g + 1) * w])
                        nc.sync.dma_start(
                            out=out[ts(sc, 128),
                                    jt * 512 + g * w:jt * 512 + (g + 1) * w],
                            in_=ot[:, osl])

    nc.compile()
    _dedup_ldweights(nc)
    return nc


def _dedup_ldweights(nc):
    """Remove back-to-back InstLdweights that reload the exact same weights.

    bass emits one LDWEIGHTS per matmul; within an accumulation group that
    shares the stationary operand the reloads are redundant and the HW pays
    ~108ns each (partially exposed in the matmul issue stream).  Deleting a
    reload is safe when it carries no semaphore waits/updates: the matmuls
    still increment the PE completion semaphore, so every WAR threshold
    computed by the tile scheduler is unchanged.
    """
    removed = 0
    for blk in nc.main_func.blocks:
        last_sig = None
        to_remove = []
        for ins in blk.instructions:
            t = type(ins).__name__
            if t == "InstLdweights":
                si = ins.sync_info
                has_sync = si is not None and (list(si.on_wait)
                                               or list(si.on_update))
                sig = (str(ins.ins[0]), str(ins.perf_mode),
                       str(ins.is_transpose))
                if sig == last_sig and not has_sync:
                    to_remove.append(ins)
                else:
                    last_sig = sig
            elif t == "InstMatmult" and ins.is_transpose:
                last_sig = None
        for ins in to_remove:
            blk.instructions.remove(ins)
        removed += len(to_remove)
    return removed


_NC_CACHE = None


def _get_nc():
    global _NC_CACHE
    if _NC_CACHE is None:
        _NC_CACHE = build()
    return _NC_CACHE


def make_in_maps(x, w1, b1, w2, b2):
    xf = np.ascontiguousarray(x.reshape(M, D).T)          # [D, M]
    # w1r[p, ic, h] = w1[h, ic*128+p]
    w1r_h = np.ascontiguousarray(w1.T.reshape(N_IC, 128, H).transpose(1, 0, 2))
    w2t_h = np.ascontiguousarray(w2.T)                    # [H, D]
    b1a_h = np.ascontiguousarray(b1.reshape(N_OC, 128).T)  # [128, 32]
    b2m_h = np.ascontiguousarray(np.broadcast_to(b2.reshape(1, D),
                                               (128, D)))
    in_maps = []
    for c in range(NCORES):
        in_maps.append({
            "xts": np.ascontiguousarray(xf[:, c * S:(c + 1) * S]),
            "w1r": w1r_h,
            "w1s": np.ascontiguousarray(
                w1[c * (H // NCORES):(c + 1) * (H // NCORES), :]),
            "w2t": w2t_h,
            "w2s": np.ascontiguousarray(
                w2[c * (D // NCORES):(c + 1) * (D // NCORES), :]),
            "b1a": b1a_h,
            "b2m": b2m_h,
        })
    return in_maps


def kernel(x, w1, b1, w2, b2, _trace=False):
    nc = _get_nc()
    in_maps = make_in_maps(np.asarray(x, dtype=np.float32),
                           np.asarray(w1, dtype=np.float32),
                           np.asarray(b1, dtype=np.float32),
                           np.asarray(w2, dtype=np.float32),
                           np.asarray(b2, dtype=np.float32))
    res = run_bass_kernel_spmd(nc, in_maps, core_ids=list(range(NCORES)),
                               trace=_trace)
    full = np.concatenate([res.results[c]["out"] for c in range(NCORES)],
                          axis=0)
    out = full.reshape(B, T, D)
    if _trace:
        kernel.last_results = res
    return out


# revision 25
# speedup vs baseline: 1.0276x; 1.0276x over previous
"""HMQ-quantized MLP (fc1 -> exact GELU -> fc2) on 8 TRN2 NeuronCores.

Strategy: data-parallel over the 16384 token rows (2048 rows/core).
The int8 fake-quant values are integers in [-127, 127], exactly representable
in bf16, and all dot-product partial sums stay far below 2^24 -- so the
dequantized GEMMs are computed EXACTLY as bf16 integer matmuls on the PE
array with fp32 PSUM accumulation, then scaled by s_a*s_w.  Rounding uses
the +/-1.5*2^23 magic-constant trick (matches jnp.round round-half-even).

All quantization reads f32 sources (any bf16 pre-rounding of continuous
values flips ~3% of the round-to-int decisions and costs ~1% rel-err per
tensor -- measured, not acceptable).  g stages to DRAM in f32.

Scheduling (trace-driven; effective PE clock is power-capped to ~1.95GHz
for sustained matmul streams, so everything else hides behind that):
 - x shard stays in SBUF after the absmax pass (no re-read); the absmax
   DMA stream rides the sync queue ALONE (w shards first, then x; early
   fc1 prefetches are time-gated past the doorbell because the tile
   scheduler otherwise hoists them into this stream -- measured), so the
   scale doorbell rings ~52us in, before the framework's startup barrier
   releases; AllGather #1 starts the moment its firmware frees up.
 - collective trigger payloads ride the idle scalar HWDGE queue: their
   completion gates the doorbell write and must not sit behind bulk
   traffic (costs ~7us on a busy queue -- measured).
 - quantize is a 3-engine pipeline in fc1 consumption order: per ic
   chunk, ACT does the x h0 half + a w1-batch-1 slice, Pool does the x
   h1 multiply-add (its bf16-cast SUB is software-slow: 14.7us), DVE
   does the w1-batch-0 slice + the h1 finish.  fc1's first two windows
   run ic-outer with h0-half row tiles first -- the PE consumes each
   chunk the moment it exists and never outruns the quantizers.
 - gelu outputs stage per-oc in contiguous [128,2048] tiles: one DVE
   absmax reduce + one gT write per oc (4x fewer DVE/DMA ops); the last
   window keeps per-st reduces and defers its gT writes past the
   doorbell so the fc1 tail has no bulk DMA in flight.
 - g absmax partials pre-combine at window 14; w2 load+quant (Pool+DVE)
   finishes by window 12.  After the last gelu only an 8-column reduce,
   a cross-partition max and a 16B DMA precede the AllGather #2 trigger.
 - the first fc2 gq tiles quantize in eighths alternating Scalar/Vector
   so fc2's first LDWEIGHTS issues ~4us after the collective lands; the
   final row-chunk drains in 256-col pieces to shorten the kernel tail.
"""

import numpy as np

import concourse.bass as bass
import concourse.mybir as mybir
import concourse.tile as tile
from concourse import bacc, bass_isa
from concourse.bass_utils import run_bass_kernel_spmd

F32 = mybir.dt.float32
BF16 = mybir.dt.bfloat16
ts = bass.ts

C_MAGIC = 1.5 * 2**23  # round-to-nearest-even for |v| < 2^22
QMAX = 127.0

NCORES = 8
B, T, D, H = 4, 4096, 1024, 4096
M = B * T            # 16384 total rows
S = M // NCORES      # 2048 rows per core

N_IC = D // 128      # 8  contraction chunks for fc1
N_OC = H // 128      # 32 output chunks for fc1 (hidden)
N_ST = S // 512      # 4  row tiles of 512
N_SC = S // 128      # 16 row chunks of 128
N_NC = H // 128      # 32 contraction chunks for fc2
N_JT = D // 512      # 2  output col tiles for fc2

Copy = mybir.ActivationFunctionType.Copy
Gelu = mybir.ActivationFunctionType.Gelu
X_AX = mybir.AxisListType.X
MAX = mybir.AluOpType.max
MULT = mybir.AluOpType.mult
SUB = mybir.AluOpType.subtract
ADD = mybir.AluOpType.add

# w2 chunk quantization schedule: all 32 chunks done by window 12 so the
# fc1 tail carries no bulk DMA (chunk 0 is prefetched at startup).
W2SCHED = {}
_hc = 0
for _w in range(1, 13):
    _n = 3 if _w <= 8 else 2
    W2SCHED[_w] = list(range(_hc, _hc + _n))
    _hc += _n
assert _hc == 32


def build():
    nc = bacc.Bacc("TRN2", target_bir_lowering=False, debug=False,
                   num_devices=NCORES)

    xts = nc.dram_tensor("xts", [D, S], F32, kind="ExternalInput")
    w1r = nc.dram_tensor("w1r", [128, N_IC, H], F32, kind="ExternalInput")
    w1s = nc.dram_tensor("w1s", [H // NCORES, D], F32, kind="ExternalInput")
    w2t = nc.dram_tensor("w2t", [H, D], F32, kind="ExternalInput")
    w2s = nc.dram_tensor("w2s", [D // NCORES, H], F32, kind="ExternalInput")
    b1a = nc.dram_tensor("b1a", [128, N_OC], F32, kind="ExternalInput")
    b2m = nc.dram_tensor("b2m", [128, D], F32, kind="ExternalInput")
    out = nc.dram_tensor("out", [S, D], F32, kind="ExternalOutput")

    with tile.TileContext(nc) as tc:
        with (
            tc.tile_pool(name="misc", bufs=1) as misc,
            tc.tile_pool(name="big", bufs=1) as bigp,
            tc.tile_pool(name="xq", bufs=1) as xqp,
            tc.tile_pool(name="w1c", bufs=3) as w1sp,
            tc.tile_pool(name="w1q", bufs=2) as w1qp,
            tc.tile_pool(name="w2c", bufs=3) as w2sp,
            tc.tile_pool(name="gout", bufs=3) as goutp,
            tc.tile_pool(name="gts", bufs=2) as gtsp,
            tc.tile_pool(name="gq", bufs=2) as gqp,
            tc.tile_pool(name="outp", bufs=2) as outp,
            tc.tile_pool(name="psum", bufs=8, space="PSUM") as psump,
            tc.tile_pool(name="dram", bufs=1, space="DRAM") as dramp,
        ):
            # ---------------- persistent DRAM intermediates ----------------
            gT = dramp.tile([H, S], F32, tag="gT")
            cc1_in = dramp.tile([1, 4], F32, tag="cc1i")
            cc1_out = dramp.tile([NCORES, 4], F32, tag="cc1o")
            cc2_in = dramp.tile([1, 4], F32, tag="cc2i")
            cc2_out = dramp.tile([NCORES, 4], F32, tag="cc2o")

            # b1 blocked on host; small, rides the scalar HWDGE queue early
            b1sb = misc.tile([128, N_OC], F32, tag="b1sb")
            nc.scalar.dma_start(out=b1sb, in_=b1a[:, :])

            # garow cols 1..3 are never meaningful; memset once so the
            # cross-partition reduce doesn't read uninitialized SBUF.
            garow = misc.tile([128, 4], F32, tag="garow")
            nc.vector.memset(garow, 0.0)

            # dummy Gelu up front: the ACT table set covering Gelu+Copy
            # loads now (~1.3us) instead of inside fc1's first window
            nc.scalar.activation(garow[:, 3:4], garow[:, 3:4], Gelu,
                                 bias=0.0, scale=1.0)

            # ------------- local abs-max pass; x STAYS in SBUF -------------
            # All absmax DMAs ride the sync queue back-to-back (358 GB/s).
            # W shards go FIRST: their reduces (incl. the slower XY form and
            # the ws-pool rotation) then hide under the x DMA stream, and
            # the critical path ends with the last x reduce chasing its DMA.
            # part1 cols: 0..7 x | 8..9 w1 | 10..11 w2 | 12..13 x7 halves
            xf = bigp.tile([128, N_IC, S], F32, tag="big", name="xf32")
            part1 = misc.tile([128, 14], F32, tag="part1")
            # w1 shard [512, 1024] -> 2 chunks [128, 2, 1024]
            # (w-chunks borrow the gout staging buffers: same size, and the
            # staging lifetime starts long after the absmax pass retires)
            for c in range(2):
                wc = goutp.tile([128, 2, 1024], F32, tag="gout",
                                name=f"w1m{c}")
                nc.sync.dma_start(
                    out=wc,
                    in_=w1s[c * 256:(c + 1) * 256, :].rearrange(
                        "(a p) d -> p a d", p=128))
                nc.vector.tensor_reduce(out=part1[:, 8 + c:9 + c], in_=wc,
                                        axis=mybir.AxisListType.XY,
                                        op=MAX, apply_absolute_value=True)
            # w2 shard [128, 4096] -> 2 chunks [128, 2048]
            for c in range(2):
                wc = goutp.tile([128, 2048], F32, tag="gout",
                                name=f"w2m{c}")
                nc.sync.dma_start(out=wc,
                                  in_=w2s[:, c * 2048:(c + 1) * 2048])
                nc.vector.tensor_reduce(out=part1[:, 10 + c:11 + c], in_=wc,
                                        axis=X_AX,
                                        op=MAX, apply_absolute_value=True)
            for ic in range(N_IC - 1):
                nc.sync.dma_start(out=xf[:, ic, :],
                                  in_=xts[ic * 128:(ic + 1) * 128, :])
                nc.vector.tensor_reduce(out=part1[:, ic:ic + 1],
                                        in_=xf[:, ic, :], axis=X_AX,
                                        op=MAX, apply_absolute_value=True)
            # last chunk lands and reduces in halves: shorter reduce tail
            lc = N_IC - 1
            for hf in range(2):
                nc.sync.dma_start(
                    out=xf[:, lc, hf * 1024:(hf + 1) * 1024],
                    in_=xts[lc * 128:(lc + 1) * 128,
                            hf * 1024:(hf + 1) * 1024])
                nc.vector.tensor_reduce(
                    out=part1[:, 12 + hf:13 + hf],
                    in_=xf[:, lc, hf * 1024:(hf + 1) * 1024], axis=X_AX,
                    op=MAX, apply_absolute_value=True)
            nc.vector.tensor_reduce(out=part1[:, 7:8], in_=part1[:, 12:14],
                                    axis=X_AX, op=MAX)

            # combine partials -> [x, w1, w2, w2] cols of arow
            arow = misc.tile([128, 4], F32, tag="arow")
            nc.vector.tensor_reduce(out=arow[:, 0:1], in_=part1[:, 0:8],
                                    axis=X_AX, op=MAX)
            nc.vector.tensor_reduce(out=arow[:, 1:2], in_=part1[:, 8:10],
                                    axis=X_AX, op=MAX)
            nc.vector.tensor_reduce(out=arow[:, 2:3], in_=part1[:, 10:12],
                                    axis=X_AX, op=MAX)
            nc.vector.tensor_copy(arow[:, 3:4], arow[:, 2:3])
            armax = misc.tile([128, 4], F32, tag="armax")
            nc.gpsimd.partition_all_reduce(armax, arow, channels=128,
                                           reduce_op=bass_isa.ReduceOp.max)

            # ------------- AllGather #1 -> global Mx, Mw1, Mw2 -------------
            # Doorbell rings ~50us in, before the startup barrier releases;
            # the AG then starts the moment the barrier ends.  The trigger
            # payload rides the idle scalar HWDGE queue so its completion
            # (which the doorbell write waits on) is ~1us, never queued
            # behind bulk traffic.
            nc.scalar.dma_start(out=cc1_in, in_=armax[0:1, :])
            nc.gpsimd.collective_compute(
                "AllGather", mybir.AluOpType.bypass,
                replica_groups=[list(range(NCORES))],
                ins=[cc1_in.opt()], outs=[cc1_out.opt()])

            # Startup prefetch rides the GPSIMD queue gated past the
            # doorbell (the tile scheduler would otherwise hoist these 3MB
            # ahead of the absmax stream and starve it -- measured).
            w1cs = []
            w2pre = {}
            with tc.tile_wait_until(0.054):
                for ocb in range(2):
                    w1c = w1sp.tile([128, N_IC, 256], F32, tag="w1c",
                                    name=f"w1c{ocb}")
                    nc.gpsimd.dma_start(
                        out=w1c, in_=w1r[:, :, ocb * 256:(ocb + 1) * 256])
                    w1cs.append(w1c)
                w2c0 = w2sp.tile([128, D], F32, tag="w2c", name="w2c0")
                nc.gpsimd.dma_start(out=w2c0, in_=w2t[0:128, :])
                w2pre[0] = w2c0
                b2r = misc.tile([128, D], F32, tag="b2r")
                nc.gpsimd.dma_start(out=b2r, in_=b2m[:, :])

            # readback on the scalar HWDGE queue (ACT idle until quantize)
            g1g = misc.tile([NCORES, 4], F32, tag="g1g")
            nc.scalar.dma_start(out=g1g, in_=cc1_out[:, :])
            g1m = misc.tile([NCORES, 4], F32, tag="g1m")
            nc.gpsimd.partition_all_reduce(g1m, g1g, channels=NCORES,
                                           reduce_op=bass_isa.ReduceOp.max)
            g1 = misc.tile([128, 4], F32, tag="g1")
            nc.gpsimd.partition_broadcast(g1, g1m)

            # scl cols: 0 sx | 1 inv_sx | 2 sw1 | 3 inv_sw1 | 4 sw2 |
            #           5 inv_sw2 | 6 d1
            scl = misc.tile([128, 8], F32, tag="scl")
            for i in range(3):
                nc.vector.tensor_scalar(out=scl[:, 2 * i:2 * i + 1],
                                        in0=g1[:, i:i + 1],
                                        scalar1=1e-8, scalar2=1.0 / QMAX,
                                        op0=MAX, op1=MULT)
                nc.vector.reciprocal(scl[:, 2 * i + 1:2 * i + 2],
                                     scl[:, 2 * i:2 * i + 1])
            nc.vector.tensor_mul(scl[:, 6:7], scl[:, 0:1], scl[:, 2:3])

            # ---- quantize: 3-engine split in fc1 consumption order --------
            # Per ic chunk: DVE slices w1 batch-0; ACT does the h0 half
            # (both magic-round passes); Pool does the h1 multiply-add
            # (f32->f32 runs at line rate there; its bf16-cast SUB is
            # software-slow -- measured 14.7us -- so DVE finishes h1).
            # w1 batch-1 quantizes on DVE mid-stream, ready before window 1.
            w1q0 = w1qp.tile([128, N_IC, 256], BF16, tag="w1q", name="w1q0")
            xqT = xqp.tile([128, N_IC, S], BF16, tag="xq")
            w1q1 = w1qp.tile([128, N_IC, 256], BF16, tag="w1q", name="w1q1")
            for ic in range(N_IC):
                # DVE: w1 batch-0 slice + x h1 finish
                w1cf = w1cs[0][:, ic, :]
                nc.vector.tensor_scalar(out=w1cf, in0=w1cf,
                                        scalar1=scl[:, 3:4], scalar2=C_MAGIC,
                                        op0=MULT, op1=ADD)
                nc.vector.tensor_scalar(out=w1q0[:, ic, :], in0=w1cf,
                                        scalar1=C_MAGIC, scalar2=None,
                                        op0=SUB)
                nsp = 2 if ic == 0 else 1  # split ic0-h0 for a fast first MM
                for q in range(nsp):
                    w = 1024 // nsp
                    xh = xf[:, ic, q * w:(q + 1) * w]
                    qh = xqT[:, ic, q * w:(q + 1) * w]
                    nc.scalar.activation(xh, xh, Copy,
                                         bias=C_MAGIC, scale=scl[:, 1:2])
                    nc.scalar.activation(qh, xh, Copy,
                                         bias=-C_MAGIC, scale=1.0)
                # ACT also quantizes w1 batch-1 per-ic (it has headroom;
                # a bulk DVE pass here stalled the window-0 xq cadence)
                w1c1f = w1cs[1][:, ic, :]
                nc.scalar.activation(w1c1f, w1c1f, Copy,
                                     bias=C_MAGIC, scale=scl[:, 3:4])
                nc.scalar.activation(w1q1[:, ic, :], w1c1f, Copy,
                                     bias=-C_MAGIC, scale=1.0)
                xh = xf[:, ic, 1024:2048]
                qh = xqT[:, ic, 1024:2048]
                nc.gpsimd.tensor_scalar(out=xh, in0=xh,
                                        scalar1=scl[:, 1:2],
                                        scalar2=C_MAGIC,
                                        op0=MULT, op1=ADD)
                nc.vector.tensor_scalar(out=qh, in0=xh,
                                        scalar1=C_MAGIC, scalar2=None,
                                        op0=SUB)

            def quant_w1(ocb):
                # bulk DVE quantize for batches 1..15 (batch 0 was sliced)
                w1c = w1cs[ocb]
                w1q = w1qp.tile([128, N_IC, 256], BF16, tag="w1q",
                                name=f"w1q{ocb}")
                w1cf = w1c.rearrange("p a b -> p (a b)")
                nc.vector.tensor_scalar(out=w1cf, in0=w1cf,
                                        scalar1=scl[:, 3:4], scalar2=C_MAGIC,
                                        op0=MULT, op1=ADD)
                nc.vector.tensor_scalar(
                    out=w1q.rearrange("p a b -> p (a b)"), in0=w1cf,
                    scalar1=C_MAGIC, scalar2=None, op0=SUB)
                return w1q

            w1qs = [w1q0, w1q1]

            # ---------------- fc1: h^T = w1q @ xq^T, gelu, stage g^T -------
            # w2qT reuses xf's SBUF slot (same pool tag, same byte size);
            # its first write WARs on the last x-quant read.
            w2qT = bigp.tile([128, N_NC, D], BF16, tag="big", name="w2qT")
            # gpart cols 0..29: one absmax per oc 0..29 (single [128,2048]
            # reduce per staged oc); cols 32..39: per-st partials for oc
            # 30/31 so the doorbell chain overlaps the last gelu drain.
            gpart = misc.tile([128, 40], F32, tag="gpart")
            gpre = misc.tile([128, 2], F32, tag="gpre")
            gstages = []
            for ocb in range(N_OC // 2):
                if ocb + 2 <= N_OC // 2 - 1:
                    w1n = w1sp.tile([128, N_IC, 256], F32, tag="w1c",
                                    name=f"w1c{ocb + 2}")
                    # windows 0/1: gate the prefetch past the doorbell so
                    # it can't steal absmax-stream bandwidth
                    with tc.tile_wait_until(0.055, enable=(ocb < 2)):
                        nc.sync.dma_start(
                            out=w1n,
                            in_=w1r[:, :, (ocb + 2) * 256:(ocb + 3) * 256])
                    w1cs.append(w1n)
                if 2 <= ocb + 1 <= N_OC // 2 - 1:
                    w1qs.append(quant_w1(ocb + 1))
                # w2 load+quant, all done by window 12; the f32 multiply-add
                # pass rides the otherwise-idle Pool engine
                for hc in W2SCHED.get(ocb, ()):
                    if hc in w2pre:
                        w2c = w2pre[hc]
                    else:
                        w2c = w2sp.tile([128, D], F32, tag="w2c",
                                        name=f"w2c{hc}")
                        with tc.tile_wait_until(0.055, enable=(ocb < 2)):
                            nc.sync.dma_start(out=w2c,
                                              in_=w2t[ts(hc, 128), :])
                    nc.gpsimd.tensor_scalar(out=w2c, in0=w2c,
                                            scalar1=scl[:, 5:6],
                                            scalar2=C_MAGIC,
                                            op0=MULT, op1=ADD)
                    nc.vector.tensor_scalar(out=w2qT[:, hc, :], in0=w2c,
                                            scalar1=C_MAGIC,
                                            scalar2=None, op0=SUB)
                w1q = w1qs[ocb]
                pts = [[psump.tile([128, 512], F32, tag="mm",
                                   name=f"pt{2 * ocb + j}_{st}")
                        for st in range(N_ST)] for j in range(2)]
                if ocb < 2:
                    # ic-outer: each xq chunk feeds both oc halves
                    # back-to-back, so quantization keeps pace with the PE.
                    # Within an ic, the h0-half row tiles (st 0/1, produced
                    # by ACT) run before the h1 tiles (Pool+DVE, ~1us
                    # later) -- matches production order.
                    for ic in range(N_IC):
                        for half in range(2):
                            for j in range(2):
                                for st in (2 * half, 2 * half + 1):
                                    nc.tensor.matmul(
                                        pts[j][st],
                                        lhsT=w1q[:, ic,
                                                 j * 128:(j + 1) * 128],
                                        rhs=xqT[:, ic, ts(st, 512)],
                                        start=(ic == 0),
                                        stop=(ic == N_IC - 1))
                else:
                    # j-outer: consecutive matmuls share the stationary
                    # operand (deduped LDWEIGHTS stay hidden).
                    for j in range(2):
                        for ic in range(N_IC):
                            for st in range(N_ST):
                                nc.tensor.matmul(
                                    pts[j][st],
                                    lhsT=w1q[:, ic, j * 128:(j + 1) * 128],
                                    rhs=xqT[:, ic, ts(st, 512)],
                                    start=(ic == 0), stop=(ic == N_IC - 1))
                for j in range(2):
                    oc = 2 * ocb + j
                    last = ocb >= N_OC // 2 - 1
                    stg = goutp.tile([128, 2048], F32, tag="gout",
                                     name=f"stg{oc}")
                    for st in range(N_ST):
                        gv = stg[:, st * 512:(st + 1) * 512]
                        nc.scalar.activation(gv, pts[j][st], Gelu,
                                             bias=b1sb[:, oc:oc + 1],
                                             scale=scl[:, 6:7])
                        if last:
                            # per-st partials: the absmax chain overlaps
                            # the final gelu drain
                            col = 32 + 4 * j + st
                            nc.vector.tensor_reduce(
                                out=gpart[:, col:col + 1], in_=gv,
                                axis=X_AX, op=MAX,
                                apply_absolute_value=True)
                    if last:
                        # gT write deferred past the scale doorbell so the
                        # fc1 tail has no bulk DMA in flight when the 16B
                        # trigger payload lands
                        gstages.append((oc, stg))
                    else:
                        nc.vector.tensor_reduce(
                            out=gpart[:, oc:oc + 1], in_=stg,
                            axis=X_AX, op=MAX, apply_absolute_value=True)
                        nc.sync.dma_start(out=gT[ts(oc, 128), :], in_=stg)
                if ocb == N_OC // 2 - 2:
                    # pre-combine oc 0..29 absmax partials during the tail
                    nc.vector.tensor_reduce(out=gpre[:, 0:1],
                                            in_=gpart[:, 0:30],
                                            axis=X_AX, op=MAX)

            # ---------------- AllGather #2 trigger: global Mg --------------
            nc.vector.tensor_reduce(out=gpre[:, 1:2], in_=gpart[:, 32:40],
                                    axis=X_AX, op=MAX)
            nc.vector.tensor_tensor(out=garow[:, 0:1], in0=gpre[:, 0:1],
                                    in1=gpre[:, 1:2], op=MAX)
            gamax = misc.tile([128, 4], F32, tag="gamax")
            nc.gpsimd.partition_all_reduce(gamax, garow, channels=128,
                                           reduce_op=bass_isa.ReduceOp.max)
            # trigger payload on the (now idle) scalar HWDGE queue: its
            # completion gates the doorbell write and must not sit behind
            # the gT-write / gather backlog (costs ~7us there -- measured)
            nc.scalar.dma_start(out=cc2_in, in_=gamax[0:1, :])
            nc.gpsimd.collective_compute(
                "AllGather", mybir.AluOpType.bypass,
                replica_groups=[list(range(NCORES))],
                ins=[cc2_in.opt()], outs=[cc2_out.opt()])

            # deferred last-window gT writes: queued on scalar BEHIND the
            # parked cc2_in trigger, so they execute during the AllGather
            for oc, stg in gstages:
                nc.scalar.dma_start(out=gT[ts(oc, 128), :], in_=stg)

            g2g = misc.tile([NCORES, 4], F32, tag="g2g")
            nc.scalar.dma_start(out=g2g, in_=cc2_out[:, :])
            g2m = misc.tile([NCORES, 4], F32, tag="g2m")
            nc.gpsimd.partition_all_reduce(g2m, g2g, channels=NCORES,
                                           reduce_op=bass_isa.ReduceOp.max)
            g2 = misc.tile([128, 4], F32, tag="g2")
            nc.gpsimd.partition_broadcast(g2, g2m)

            # scl2 cols: 0 sg | 1 inv_sg | 2 d2
            scl2 = misc.tile([128, 4], F32, tag="scl2")
            nc.vector.tensor_scalar(out=scl2[:, 0:1], in0=g2[:, 0:1],
                                    scalar1=1e-8, scalar2=1.0 / QMAX,
                                    op0=MAX, op1=MULT)
            nc.vector.reciprocal(scl2[:, 1:2], scl2[:, 0:1])
            nc.vector.tensor_mul(scl2[:, 2:3], scl2[:, 0:1], scl[:, 4:5])

            # ---------------- fc2: out = gq^T.T @ w2q^T --------------------
            for sc in range(N_SC):
                gqs = []
                for half in range(2):
                    gs = gtsp.tile([128, 16, 128], F32, tag="gts",
                                   name=f"gs{sc}_{half}")
                    nc.sync.dma_start(
                        out=gs,
                        in_=gT[half * 2048:(half + 1) * 2048,
                               ts(sc, 128)].rearrange("(a p) s -> p a s",
                                                      p=128))
                    gq = gqp.tile([128, 16, 128], BF16, tag="gq",
                                  name=f"gq{sc}_{half}")
                    if sc < 2:
                        # fine-grained, alternating engines: the first
                        # lhsT tile is ready well under 1us after scl2.
                        npc = 8 if (sc == 0 and half == 0) else 4
                        w = 16 // npc
                        for q in range(npc):
                            gsf = gs[:, w * q:w * (q + 1), :].rearrange(
                                "p a b -> p (a b)")
                            gqf = gq[:, w * q:w * (q + 1), :].rearrange(
                                "p a b -> p (a b)")
                            nc.scalar.activation(gsf, gsf, Copy,
                                                 bias=C_MAGIC,
                                                 scale=scl2[:, 1:2])
                            nc.vector.tensor_scalar(out=gqf, in0=gsf,
                                                    scalar1=C_MAGIC,
                                                    scalar2=None, op0=SUB)
                    else:
                        gsf = gs.rearrange("p a b -> p (a b)")
                        nc.scalar.activation(gsf, gsf, Copy, bias=C_MAGIC,
                                             scale=scl2[:, 1:2])
                        nc.vector.tensor_scalar(
                            out=gq.rearrange("p a b -> p (a b)"), in0=gsf,
                            scalar1=C_MAGIC, scalar2=None, op0=SUB)
                    gqs.append(gq)
                pos = [psump.tile([128, 512], F32, tag="mm",
                                  name=f"po{sc}_{jt}")
                       for jt in range(N_JT)]
                for nn in range(N_NC):
                    for jt in range(N_JT):
                        nc.tensor.matmul(pos[jt],
                                         lhsT=gqs[nn // 16][:, nn % 16, :],
                                         rhs=w2qT[:, nn, ts(jt, 512)],
                                         start=(nn == 0),
                                         stop=(nn == N_NC - 1))
                # last row-chunk drains in 256-col pieces: shorter kernel
                # tail after the final matmul
                nseg = 2 if sc == N_SC - 1 else 1
                for jt in range(N_JT):
                    ot = outp.tile([128, 512], F32, tag="ot",
                                   name=f"ot{sc}_{jt}")
                    for g in range(nseg):
                        w = 512 // nseg
                        osl = slice(g * w, (g + 1) * w)
                        nc.scalar.activation(ot[:, osl], pos[jt][:, osl],
                                             Copy, bias=0.0,
                                             scale=scl2[:, 2:3])
                        nc.vector.tensor_add(ot[:, osl], ot[:, osl],
                                             b2r[:, jt * 512 + g * w:
                                                 jt * 512 + (g + 1) * w])
                        nc.sync.dma_start(
                            out=out[ts(sc, 128),
                                    jt * 512 + g * w:jt * 512 + (g + 1) * w],
                            in_=ot[:, osl])

    nc.compile()
    _dedup_ldweights(nc)
    return nc


def _dedup_ldweights(nc):
    """Remove back-to-back InstLdweights that reload the exact same weights.

    bass emits one LDWEIGHTS per matmul; within an accumulation group that
    shares the stationary operand the reloads are redundant and the HW pays
    ~108ns each (partially exposed in the matmul issue stream).  Deleting a
    reload is safe when it carries no semaphore waits/updates: the matmuls
    still increment the PE completion semaphore, so every WAR threshold
    computed by the tile scheduler is unchanged.
    """
    removed = 0
    for blk in nc.main_func.blocks:
        last_sig = None
        to_remove = []
        for ins in blk.instructions:
            t = type(ins).__name__
            if t == "InstLdweights":
                si = ins.sync_info
                has_sync = si is not None and (list(si.on_wait)
                                               or list(si.on_update))
                sig = (str(ins.ins[0]), str(ins.perf_mode),
                       str(ins.is_transpose))
                if sig == last_sig and not has_sync:
                    to_remove.append(ins)
                else:
                    last_sig = sig
            elif t == "InstMatmult" and ins.is_transpose:
                last_sig = None
        for ins in to_remove:
            blk.instructions.remove(ins)
        removed += len(to_remove)
    return removed


_NC_CACHE = None


def _get_nc():
    global _NC_CACHE
    if _NC_CACHE is None:
        _NC_CACHE = build()
    return _NC_CACHE


def make_in_maps(x, w1, b1, w2, b2):
    xf = np.ascontiguousarray(x.reshape(M, D).T)          # [D, M]
    # w1r[p, ic, h] = w1[h, ic*128+p]
    w1r_h = np.ascontiguousarray(w1.T.reshape(N_IC, 128, H).transpose(1, 0, 2))
    w2t_h = np.ascontiguousarray(w2.T)                    # [H, D]
    b1a_h = np.ascontiguousarray(b1.reshape(N_OC, 128).T)  # [128, 32]
    b2m_h = np.ascontiguousarray(np.broadcast_to(b2.reshape(1, D),
                                               (128, D)))
    in_maps = []
    for c in range(NCORES):
        in_maps.append({
            "xts": np.ascontiguousarray(xf[:, c * S:(c + 1) * S]),
            "w1r": w1r_h,
            "w1s": np.ascontiguousarray(
                w1[c * (H // NCORES):(c + 1) * (H // NCORES), :]),
            "w2t": w2t_h,
            "w2s": np.ascontiguousarray(
                w2[c * (D // NCORES):(c + 1) * (D // NCORES), :]),
            "b1a": b1a_h,
            "b2m": b2m_h,
        })
    return in_maps


def kernel(x, w1, b1, w2, b2, _trace=False):
    nc = _get_nc()
    in_maps = make_in_maps(np.asarray(x, dtype=np.float32),
                           np.asarray(w1, dtype=np.float32),
                           np.asarray(b1, dtype=np.float32),
                           np.asarray(w2, dtype=np.float32),
                           np.asarray(b2, dtype=np.float32))
    res = run_bass_kernel_spmd(nc, in_maps, core_ids=list(range(NCORES)),
                               trace=_trace)
    full = np.concatenate([res.results[c]["out"] for c in range(NCORES)],
                          axis=0)
    out = full.reshape(B, T, D)
    if _trace:
        kernel.last_results = res
    return out
